# revision 21
# baseline (speedup 1.0000x reference)
"""SupCon cluster-memory loss kernel for 8 TRN2 NeuronCores.

Problem: 4 SupCon losses (rgb/ir anchors x rgb/ir memory banks).
  logits = l2norm(x) @ mem.T / T   [256, 8192]
  loss   = -mean_i[ (sum_j mask*log_prob) / max(sum_j mask, 1) ]

Sharding: memory banks split column-wise (N=8192 -> 1024 per core),
anchor batches replicated.  Each core computes, for its N-shard and all
4 (anchor, bank) combos, sumexp[i] = sum_j exp(logits_ij/T - shift_b)
via bf16 matmuls + ScalarE Exp with fused row-accumulate.

The positives term only touches the <=few memory rows whose prototype
label matches each anchor (exactly one for permutation labels).  The
host gathers those rows (index bookkeeping only, G_i = sum of matching
rows) and each of cores 0..3 computes one combo's positive dot products
pos_i = x_i . G_i on-device (VectorE mult+reduce over D).

Host combine: LSE_i = shift_b + log(sum_cores sumexp_i),
mlpp_i = (pos_i/T - cnt_i*LSE_i)/max(cnt_i,1), loss = -mean_i mlpp_i.

shift_b = max_row_norm(bank_b)/T bounds |logits| (anchors unit-norm):
exp args <= 0, no overflow; unit-norm banks keep the range e^-29..1.
"""

from contextlib import ExitStack

import ml_dtypes
import numpy as np

import concourse.bacc as bacc
import concourse.bass as bass
import concourse.mybir as mybir
import concourse.tile as tile
from concourse.bass_utils import run_bass_kernel_spmd

BF16_NP = ml_dtypes.bfloat16

B = 256          # anchor batch per modality
N = 8192         # memory bank rows
D = 768          # feature dim
NCORES = 8
NS = N // NCORES     # 1024 bank rows per core
KT = D // 128        # 6 contraction tiles
MT = B // 128        # 2 anchor partition tiles
NT = NS // 512       # 2 psum free-dim tiles
SUPCON_T = 0.07

F32 = mybir.dt.float32
BF16 = mybir.dt.bfloat16
FP8 = mybir.dt.float8e4
FP8_NP = ml_dtypes.float8_e4m3
FP8_SCALE = 16.0

_NC_CACHE = {}


def _build_nc():
    nc = bacc.Bacc("TRN2", target_bir_lowering=False, debug=False,
                   num_devices=NCORES)

    # Per-core DRAM inputs (host pre-transposed to K-major layouts).
    xT = nc.dram_tensor("xT", [128, KT, 2, B], FP8, kind="ExternalInput").ap()
    memT = nc.dram_tensor("memT", [2, 128, KT, NS], FP8, kind="ExternalInput").ap()
    nshift_h = nc.dram_tensor("nshift", [2], F32, kind="ExternalInput")
    # Positive pairs, anchor-major: this core's combo (cores 0-3; 4-7 get
    # zeros and their pos output is ignored).
    posx = nc.dram_tensor("posx", [128, D], BF16, kind="ExternalInput").ap()
    posg = nc.dram_tensor("posg", [128, D], BF16, kind="ExternalInput").ap()
    # Outputs: res_s col = mt*4 + c (combo c = a*2+b); res_p col = mt.
    res_s = nc.dram_tensor("res_s", [128, 16], F32, kind="ExternalOutput").ap()
    res_p = nc.dram_tensor("res_p", [128, 1], F32, kind="ExternalOutput").ap()

    with tile.TileContext(nc) as tc, ExitStack() as ctx:
        const = ctx.enter_context(tc.tile_pool(name="const", bufs=1))
        wpool = ctx.enter_context(tc.tile_pool(name="wpool", bufs=1))
        mpool = ctx.enter_context(tc.tile_pool(name="mpool", bufs=1))
        pospool = ctx.enter_context(tc.tile_pool(name="pospool", bufs=1))
        psum = ctx.enter_context(tc.tile_pool(name="psum", bufs=8, space="PSUM"))
        ep = ctx.enter_context(tc.tile_pool(name="ep", bufs=4))
        outp = ctx.enter_context(tc.tile_pool(name="outp", bufs=1))

        rs = outp.tile([128, 16], F32, tag="rs", name="rs")
        rp = outp.tile([128, 1], F32, tag="rp", name="rp")

        # Anchor features and bank shards, all p-major in DRAM so every DMA
        # chunk moves 3-6KB contiguous runs per partition (small descriptors
        # choke the SDMA engines at ~10GB/s).  kt 0-2 chunks land first, in
        # the accumulation loop's consumption order; the two HWDGE rings
        # split the load.
        x_all = wpool.tile([128, KT, 2, B], FP8, tag="x_all", name="x_all")
        mbig = []
        for b in range(2):
            t = mpool.tile([128, KT, NS], FP8, tag=f"m{b}", name=f"m{b}")
            mbig.append(t)
        shift_t = const.tile([128, 2], F32, tag="shift", name="shift")

        nc.sync.dma_start(out=shift_t,
                          in_=bass.AP(tensor=nshift_h, offset=0,
                                      ap=[[0, 128], [1, 2]]))
        nc.sync.dma_start(out=x_all[:, 0:2], in_=xT[:, 0:2])
        nc.scalar.dma_start(out=mbig[1][:, 0:2], in_=memT[1][:, 0:2])
        nc.sync.dma_start(out=mbig[0][:, 0:2], in_=memT[0][:, 0:2])
        nc.scalar.dma_start(out=x_all[:, 2:6], in_=xT[:, 2:6])
        nc.sync.dma_start(out=mbig[0][:, 2:4], in_=memT[0][:, 2:4])
        nc.scalar.dma_start(out=mbig[1][:, 2:4], in_=memT[1][:, 2:4])
        nc.sync.dma_start(out=mbig[0][:, 4:6], in_=memT[0][:, 4:6])
        nc.scalar.dma_start(out=mbig[1][:, 4:6], in_=memT[1][:, 4:6])
        # This core's positive pair rows (combo c//2, anchor half c%2) ride
        # the gpsimd SWDGE ring; they are only needed by the VectorE dots.
        pxt = pospool.tile([128, D], BF16, tag="px", name="pxt")
        pgt = pospool.tile([128, D], BF16, tag="pg", name="pgt")
        nc.gpsimd.dma_start(out=pxt, in_=posx)
        nc.gpsimd.dma_start(out=pgt, in_=posg)

        # PE warmup during the DMA window: garbage matmuls into a scratch
        # bank release the HAM clock throttle before the real stream starts.
        wu_w = const.tile([128, 128], BF16, tag="wu_w", name="wu_w")
        wu_r = const.tile([128, 512], BF16, tag="wu_r", name="wu_r")
        wu_p = psum.tile([128, 512], F32, tag="acc", name="wu_p")
        nc.vector.memset(wu_w, 0.0)
        nc.vector.memset(wu_r, 0.0)
        for _ in range(4):
            nc.tensor.matmul(wu_p[:], wu_w[:], wu_r[:], start=True, stop=True)

        # Positive dot products (VectorE; tiny).
        pm = ep.tile([128, D], F32, tag="pm", name="pm")
        nc.vector.tensor_tensor(out=pm, in0=pxt, in1=pgt,
                                op=mybir.AluOpType.mult)
        nc.vector.tensor_reduce(out=rp, in_=pm,
                                axis=mybir.AxisListType.X,
                                op=mybir.AluOpType.add)

        # Two mt-waves; each uses all 8 PSUM banks as single-bank accs (a
        # kt chunk feeds 8 matmuls ~1.9us, matching the ~1.6us/kt delivery
        # rate; single-bank accs avoid psum-queue cycling).
        for mt in range(MT):
            acc = {}
            for a in range(2):
                for b in range(2):
                    for nt in range(NT):
                        acc[a, b, nt] = psum.tile([128, 512], F32, tag="acc",
                                                  name=f"acc{mt}_{a}{b}{nt}")
            for kp in range(KT // 2):
                for a in range(2):
                    lhsT = x_all[:, 2 * kp:2 * kp + 2, a, mt * 128:(mt + 1) * 128]
                    for b in range(2):
                        for nt in range(NT):
                            nc.tensor.matmul(
                                acc[a, b, nt][:],
                                lhsT,
                                mbig[b][:, 2 * kp:2 * kp + 2,
                                        nt * 512:(nt + 1) * 512],
                                start=(kp == 0), stop=(kp == KT // 2 - 1),
                                perf_mode=mybir.MatmulPerfMode.DoubleRow)
            for a in range(2):
                for b in range(2):
                    c = a * 2 + b
                    for nt in range(NT):
                        col = mt * 8 + c * 2 + nt
                        ex = ep.tile([128, 512], F32, tag="ex",
                                     name=f"ex{mt}{nt}_{c}")
                        nc.scalar.activation(
                            out=ex, in_=acc[a, b, nt][:],
                            func=mybir.ActivationFunctionType.Exp,
                            bias=shift_t[:, b:b + 1],
                            scale=1.0 / (SUPCON_T * FP8_SCALE * FP8_SCALE))
                        nc.vector.tensor_reduce(
                            out=rs[:, col:col + 1], in_=ex,
                            axis=mybir.AxisListType.X,
                            op=mybir.AluOpType.add)

        nc.sync.dma_start(out=res_s, in_=rs)
        nc.sync.dma_start(out=res_p, in_=rp)

    nc.compile()
    return nc


def get_nc():
    if "nc" not in _NC_CACHE:
        _NC_CACHE["nc"] = _build_nc()
    return _NC_CACHE["nc"]


def _l2norm(x):
    n = np.linalg.norm(x, axis=-1, keepdims=True)
    return x / np.maximum(n, 1e-12)


def _gather_positives(feats_b, lab_a, mlab_b):
    """G[i] = sum of bank rows whose prototype label == lab_a[i].

    Pure index bookkeeping for permutation labels (single match); falls
    back to a scatter-add for general labels."""
    G = np.zeros((B, D), np.float32)
    if np.unique(mlab_b).size == mlab_b.size:
        inv = np.full(1 << 14, -1, np.int64)
        inv[mlab_b] = np.arange(mlab_b.size)
        idx = inv[np.clip(lab_a, 0, (1 << 14) - 1)]
        valid = idx >= 0
        G[valid] = feats_b[idx[valid]]
    else:
        by_label = np.zeros((1 << 14, D), np.float32)
        np.add.at(by_label, mlab_b, feats_b)
        G[:] = by_label[np.clip(lab_a, 0, (1 << 14) - 1)]
    return G


def make_in_maps(inputs_rgb, inputs_ir, targets_rgb, targets_ir,
                 features_rgb, features_ir,
                 prototype_labels_rgb, prototype_labels_ir):
    x = [_l2norm(np.asarray(inputs_rgb, np.float32)),
         _l2norm(np.asarray(inputs_ir, np.float32))]
    feats = [np.asarray(features_rgb, np.float32),
             np.asarray(features_ir, np.float32)]
    lab = [np.asarray(targets_rgb).astype(np.int64),
           np.asarray(targets_ir).astype(np.int64)]
    mlab = [np.asarray(prototype_labels_rgb).astype(np.int64),
            np.asarray(prototype_labels_ir).astype(np.int64)]

    xT = np.empty([2, KT, 128, B], np.float32)
    for a in range(2):
        xT[a] = x[a].T.reshape(KT, 128, B) * FP8_SCALE
    xT = np.ascontiguousarray(xT.transpose(2, 1, 0, 3)).astype(FP8_NP)  # [128,KT,2,B]

    bank_max = [float(np.sqrt((feats[b] ** 2).sum(axis=1).max()))
                for b in range(2)]
    shift = np.array([bank_max[0] / SUPCON_T, bank_max[1] / SUPCON_T],
                     np.float64)
    nshift = (-shift).astype(np.float32)

    in_maps = []
    for c in range(NCORES):
        memT = np.empty([2, 128, KT, NS], FP8_NP)
        for b in range(2):
            memT[b] = (feats[b][c * NS:(c + 1) * NS, :].T * FP8_SCALE).reshape(
                KT, 128, NS).transpose(1, 0, 2).astype(FP8_NP)
        combo, mt = c // 2, c % 2
        a, b = combo // 2, combo % 2
        G = _gather_positives(feats[b], lab[a], mlab[b])
        posx = x[a][mt * 128:(mt + 1) * 128].astype(BF16_NP)
        posg = G[mt * 128:(mt + 1) * 128].astype(BF16_NP)
        in_maps.append({
            "xT": xT,
            "memT": memT,
            "nshift": nshift,
            "posx": posx,
            "posg": posg,
        })
    return in_maps, shift


def combine(results, shift, targets_rgb, targets_ir,
            prototype_labels_rgb, prototype_labels_ir):
    rs = np.stack([np.asarray(r["res_s"], np.float64) for r in results])
    rs = rs.reshape(NCORES, 128, MT, 4, NT).sum(axis=(0, 4))  # [128, mt, c]
    sumexp = rs.transpose(1, 0, 2).reshape(B, 4)              # i = mt*128+p

    lab = [np.asarray(targets_rgb).astype(np.int64),
           np.asarray(targets_ir).astype(np.int64)]
    mlab = [np.asarray(prototype_labels_rgb).astype(np.int64),
            np.asarray(prototype_labels_ir).astype(np.int64)]

    losses = np.zeros(4, np.float64)
    for a in range(2):
        for b in range(2):
            c = a * 2 + b
            pos = np.concatenate([
                np.asarray(results[c * 2 + mt]["res_p"], np.float64)[:, 0]
                for mt in range(MT)])
            lse = shift[b] + np.log(sumexp[:, c])
            cnt = np.bincount(mlab[b], minlength=1 << 14)[
                np.clip(lab[a], 0, (1 << 14) - 1)].astype(np.float64)
            mlpp = (pos / SUPCON_T - cnt * lse) / np.maximum(cnt, 1.0)
            losses[c] = -mlpp.mean()

    loss_contr = losses[0] + losses[3]        # (rgb,rgb) + (ir,ir)
    loss_cross = losses[1] + losses[2]        # (rgb,ir)  + (ir,rgb)
    return np.asarray([loss_contr, loss_cross], np.float32)


def run_device(in_maps, **kwargs):
    return run_bass_kernel_spmd(get_nc(), in_maps,
                                core_ids=list(range(NCORES)), **kwargs)


def kernel(inputs_rgb, inputs_ir, targets_rgb, targets_ir,
           features_rgb, features_ir,
           prototype_labels_rgb, prototype_labels_ir):
    in_maps, shift = make_in_maps(inputs_rgb, inputs_ir, targets_rgb,
                                  targets_ir, features_rgb, features_ir,
                                  prototype_labels_rgb, prototype_labels_ir)
    results = run_device(in_maps).results
    return combine(results, shift, targets_rgb, targets_ir,
                   prototype_labels_rgb, prototype_labels_ir)
